# revision 30
# baseline (speedup 1.0000x reference)
"""Trainium2 Bass kernel for nn_Block_47545287967557 (dense_cnn).

The reference module, simplified:
  - dead avgpool->linear->relu path (result unused)
  - sum over K=4 conv branches == ONE 3x3 VALID conv with weights Wc.sum(0)
    and bias bc.sum(0):  O[b,co,y,x] = sum_{ci,dy,dx} Weff[co,ci,dy,dx] *
    X[b,ci,y+dy,x+dx] + beff[co]
  X: [32,3,512,512] fp32 -> O: [32,3,510,510] fp32.

Strategy: pure data-parallel over batch across 8 NeuronCores (4 images each).
Per core the conv runs on the tensor engine as block-banded matmuls:
  contraction K = (c_in, yi) packed into 126 partitions (42-row y window)
  plus one constant-ones partition that folds the bias into the dx=0 matmul,
  output M = (c_out, yo) packed into 120 partitions (+8 zero pad to 128),
  moving N = 510 x positions; one matmul per dx shift (3, PSUM-accumulated).
  13 y-blocks per image (y0 = 0,40,...,440,470; the last overlaps rows
  470..479 with identical values).

DMA: the host casts X to fp16 and shards it directly into the matmul layout
XP[img, (c,yi)+ones, b*512+x] (the overlap-window gather is part of
sharding), halving input HBM traffic vs fp32 and eliminating the on-device
cast. The device writes output partition-major OUT[img, (c,yo), b*510+x] as
fp16; the host inverts that layout while unsharding. Every DMA moves >=13KB
contiguous per partition. PSUM->SBUF copies alternate between ScalarE
(activation Copy) and VectorE (tensor_copy) so neither engine bottlenecks.
"""

import sys

sys.path.insert(0, "/opt/trn_rl_repo")

import numpy as np

N_CORES = 8
B_PER_CORE = 4
C = 3
H = W = 512
OH = OW = 510
NBLK = 13
KP = 128         # 126 contraction rows + ones-row (bias) + zero pad row (even partition count for DMA spray)
MP = C * 40      # 120 live output partitions
MPAD = 128       # stationary columns padded for FWL
XROW = NBLK * W          # 6656 fp16 elements per input partition row
XPITCH = XROW + 64       # padded DRAM row pitch: keeps the DRAM source AP
                         # strided (stride != len) so the HWDGE spreads the
                         # per-partition descriptors across all 16 SDMA
                         # engines instead of chaining one linear region
                         # onto a single engine (~27 GiB/s)

_CACHE = {}


def _build_weights(Wc, bc):
    Weff = np.asarray(Wc, dtype=np.float32).sum(axis=0)  # [co, ci, dy, dx]
    beff = np.asarray(bc, dtype=np.float32).sum(axis=0)  # [co]
    S = np.zeros((3, KP, MPAD), dtype=np.float32)
    for dx in range(3):
        for c_in in range(C):
            for c_out in range(C):
                for yo in range(40):
                    for dy in range(3):
                        S[dx, c_in * 42 + yo + dy, c_out * 40 + yo] = Weff[c_out, c_in, dy, dx]
    # bias via the constant-ones row (partition 126), only in the dx=0 matmul;
    # partition 127 is zero padding
    for c_out in range(C):
        S[0, C * 42, c_out * 40:(c_out + 1) * 40] = beff[c_out]
    return S.astype(np.float16)


def _build_program():
    import concourse.bass as bass
    import concourse.mybir as mybir
    import concourse.tile as tile
    from concourse import bacc

    nc = bacc.Bacc("TRN2", target_bir_lowering=False, debug=False)

    XS = nc.dram_tensor("XS", [B_PER_CORE, KP, NBLK, W], mybir.dt.float16, kind="ExternalInput")
    SMAT = nc.dram_tensor("SMAT", [3, KP, MPAD], mybir.dt.float16, kind="ExternalInput")
    OUT = nc.dram_tensor("OUT", [B_PER_CORE, MP, NBLK, OW], mybir.dt.float16, kind="ExternalOutput")

    f16 = mybir.dt.float16
    f32 = mybir.dt.float32
    ident = mybir.ActivationFunctionType.Identity

    with tile.TileContext(nc) as tc:
        with (
            tc.tile_pool(name="consts", bufs=1) as consts,
            tc.tile_pool(name="xs", bufs=3) as xpool,
            tc.tile_pool(name="os", bufs=3) as opool,
            tc.tile_pool(name="ps", bufs=7, space=bass.MemorySpace.PSUM) as ppool,
            tc.tile_pool(name="pswarm", bufs=1, space=bass.MemorySpace.PSUM) as wpool,
        ):
            smat_t = []
            for d in range(3):
                st = consts.tile([KP, MPAD], f16, tag=f"smat{d}")
                # HWDGE, not gpsimd SWDGE: the SWDGE path takes ~12us to land
                # the weights, stalling the first (weight-dependent) matmul
                nc.sync.dma_start(out=st[:], in_=SMAT.ap()[d])
                smat_t.append(st)

            # warm the PE p-state during the DMA head: dummy matmuls that
            # depend only on a locally memset tile, so they start right after
            # the framework preamble and ramp the clock before real work
            wm = consts.tile([KP, OW + 2], f16, tag="warm")
            nc.vector.memset(wm[:], 0.0)
            wt = wpool.tile([MPAD, OW], f32)
            for _ in range(9):
                nc.tensor.matmul(wt[:], wm[:, 0:MPAD], wm[:, 2:OW + 2], start=True, stop=True)
            # fp8 DoubleRow timing probe (junk data, result unused): measures
            # whether HW really streams 0.5 cycles/row in DoubleRow mode
            w8 = consts.tile([KP, 2, OW + 2], mybir.dt.float8e4, tag="w8probe")
            nc.vector.memset(w8[:], 0.0)

            for img in range(B_PER_CORE):
                xb = xpool.tile([KP, NBLK, W], f16)
                ot = opool.tile([MP, NBLK, OW], f16)
                # img0: small first chunk so the first matmul starts early.
                # Loads stay on the qSP ring; stores go on qAct — per-ring FIFO
                # means a store ahead of a load would head-of-line block it.
                in_chunks = ((0, 3), (3, 8), (8, 13)) if img == 0 else ((0, 7), (7, 13))
                for b0, b1 in in_chunks:
                    nc.sync.dma_start(out=xb[:, b0:b1, :], in_=XS.ap()[img, :, b0:b1, :])
                # store in chunks, alternating HWDGE rings, so stores start
                # earlier and the final store is small (short tail)
                out_chunks = ((0, 5), (5, 9), (9, 11), (11, 13)) if img == B_PER_CORE - 1 \
                    else ((0, 5), (5, 9), (9, 13))
                for ci, (b0, b1) in enumerate(out_chunks):
                    for b in range(b0, b1):
                        pt = ppool.tile([MPAD, OW], f32)
                        for dx in range(3):
                            nc.tensor.matmul(
                                pt[:],
                                smat_t[dx][:],
                                xb[:, b, dx:dx + OW],
                                start=(dx == 0),
                                stop=(dx == 2),
                            )
                        if b % 2 == 0:
                            nc.scalar.activation(ot[:, b, :], pt[0:MP, :], ident, scale=1.0)
                        else:
                            nc.vector.tensor_copy(ot[:, b, :], pt[0:MP, :])
                    nc.scalar.dma_start(out=OUT.ap()[img, :, b0:b1, :], in_=ot[:, b0:b1, :])

            for _ in range(8):
                nc.tensor.matmul(
                    wt[:],
                    w8[:, :, 0:MPAD],
                    w8[:, :, 2:OW + 2],
                    start=True,
                    stop=True,
                    perf_mode=mybir.MatmulPerfMode.DoubleRow,
                )

    nc.compile()
    return nc


def _get_nc():
    if "nc" not in _CACHE:
        _CACHE["nc"] = _build_program()
    return _CACHE["nc"]


def run_spmd(in_maps, **kwargs):
    from concourse.bass_utils import run_bass_kernel_spmd

    nc = _get_nc()
    return run_bass_kernel_spmd(nc, in_maps, list(range(N_CORES)), **kwargs)


def make_in_maps(X, Wc, bc):
    X = np.ascontiguousarray(np.asarray(X, dtype=np.float32))
    Sb = _build_weights(Wc, bc)

    # overlap-window shard: XP[core, img, c*42+yi, b, x] = X[4*core+img, c, y0(b)+yi, x]
    Xr = X.reshape(N_CORES, B_PER_CORE, C, H, W)
    XP = np.empty((N_CORES, B_PER_CORE, KP, NBLK, W), dtype=np.float16)
    s = Xr.strides
    win = np.lib.stride_tricks.as_strided(
        Xr, shape=(N_CORES, B_PER_CORE, C, 12, 42, W),
        strides=(s[0], s[1], s[2], 40 * s[3], s[3], s[4]))
    XPc = XP[:, :, :C * 42].reshape(N_CORES, B_PER_CORE, C, 42, NBLK, W)
    XPc[:, :, :, :, 0:12, :] = win.transpose(0, 1, 2, 4, 3, 5)
    XPc[:, :, :, :, 12, :] = Xr[:, :, :, 470:512, :]
    XP[:, :, C * 42, :, :] = np.float16(1.0)  # ones row for bias matmul
    XP[:, :, C * 42 + 1:, :, :] = np.float16(0.0)  # zero pad rows

    return [
        {"XS": XP[i], "SMAT": Sb}
        for i in range(N_CORES)
    ]


def gather_output(res):
    """[core][img, (c,yo), b*510+x] -> [32, 3, 510, 510]"""
    OUTP = np.stack([res.results[i]["OUT"] for i in range(N_CORES)]).astype(np.float32)
    R = OUTP.reshape(N_CORES, B_PER_CORE, C, 40, NBLK, OW)  # OUT dram is [img, MP, NBLK, OW]
    O = np.empty((N_CORES, B_PER_CORE, C, OH, OW), dtype=np.float32)
    O[:, :, :, 0:480, :] = (
        R[:, :, :, :, 0:12, :].transpose(0, 1, 2, 4, 3, 5).reshape(N_CORES, B_PER_CORE, C, 480, OW)
    )
    O[:, :, :, 480:OH, :] = R[:, :, :, 10:40, 12, :]
    return O.reshape(N_CORES * B_PER_CORE, C, OH, OW)


def kernel(X, Wc, bc, linW, linb):
    res = run_spmd(make_in_maps(X, Wc, bc))
    return gather_output(res)
